# revision 28
# baseline (speedup 1.0000x reference)
"""DCT-attention Trainium2 kernel (8 NeuronCores, data-parallel over batch).

Reference math (per b, h):
    Qd = dct @ (Q*s);  Kd = dct @ (K*s*mask);  Vd = dct @ (V*mask)   # [M,D]
    E  = Qd @ Kd^T;  P = softmax(E, axis=-1);  ctx = P @ Vd          # [M,D]
    x  = dct^T @ ctx                                                 # [N,D]
with B,H,N,D = 8,12,2048,64, M = 256, s = D**-0.25.

Sharding: batch b -> core b (8 cores). Host folds scale into Q/K and mask into
K/V, transposes to [N, H*D], bf16-casts; matmuls run bf16 -> fp32 PSUM; output
returns bf16 and is cast to f32 on the host.

DCT parity symmetry: dct[k, N-1-i] = (-1)^k dct[k, i].  The host therefore
uploads X folded as [A; B] with A = X[:N/2] + reverse(X[N/2:]),
B = X[:N/2] - reverse(X[N/2:]), and the M axis globally reordered to
[even k | odd k] (dctT columns, dct rows, and everything downstream use the
same order; softmax/ctx are permutation-invariant).  Every projection then
contracts over N/2 instead of N — half the matmul work.

Schedule per core (single FIFO DMA queue; PE chases the arrival stream):
  DMA:  dctT(0.5MB) | q | k | v | dct(1MB)
  PE:   Q-proj (A-phase 6 groups chunk-major, then B-phase) ->
        K-proj A -> K-proj B interleaved with even-k energies (exps on ACT
        overlap) -> V-proj interleaved with odd-k energies ->
        phase B heads 0-5 -> inverse-DCT fs0 half (+ out DMA) ->
        phase B heads 6-11 -> inverse-DCT fs1 half (+ out DMA).
"""

import numpy as np
import ml_dtypes

B, H, N, D = 8, 12, 2048, 64
M = 256
HD = H * D          # 768
NH = N // 2         # 1024 folded length
NCH = NH // 128     # 8 folded chunks per parity phase
QT = 4              # chunks per input-DMA quarter (of the folded [N, HD] array)
MB = M // 128       # 2 m-blocks (even ks | odd ks)
HP = H // 2         # 6 head-pairs
FSPLIT = 2          # HD split for <=512-wide psum
FW = HD // FSPLIT   # 384
VW = D + 1          # 65: Vd columns + ones column

_BF16 = ml_dtypes.bfloat16
_CACHE = {}


def build_nc():
    import concourse.bacc as bacc
    import concourse.mybir as mybir
    import concourse.tile as tile
    from contextlib import ExitStack

    BF = mybir.dt.bfloat16
    F32 = mybir.dt.float32
    EXP = mybir.ActivationFunctionType.Exp
    COPY = mybir.ActivationFunctionType.Copy

    # Bacc (not raw Bass): its compile() runs generate_event_semaphores,
    # which legalizes to this walrus's 1-sync-wait-per-instruction limit.
    nc = bacc.Bacc()
    q_d = nc.declare_dram_parameter("q", [N, HD], BF, isOutput=False)
    k_d = nc.declare_dram_parameter("k", [N, HD], BF, isOutput=False)
    v_d = nc.declare_dram_parameter("v", [N, HD], BF, isOutput=False)
    # dctT: [n' < N/2, m] with columns [even k | odd k]:
    #   cols 0:128  = dct[2j, n']^T,  cols 128:256 = dct[2j+1, n']^T
    dctT_d = nc.declare_dram_parameter("dctT", [NH, M], BF, isOutput=False)
    # dct: [M, N/2] rows reordered [even k | odd k]; the upper output half is
    # reconstructed from the same columns via x[N-1-i] = even - odd (host
    # un-reverses those rows).
    dct_d = nc.declare_dram_parameter("dct", [M, NH], BF, isOutput=False)
    out_d = nc.declare_dram_parameter("out", [N, HD], BF, isOutput=True)

    q_r = q_d.ap().rearrange("(c p) f -> p c f", p=128)
    k_r = k_d.ap().rearrange("(c p) f -> p c f", p=128)
    v_r = v_d.ap().rearrange("(c p) f -> p c f", p=128)
    dctT_r = dctT_d.ap().rearrange("(c p) m -> p c m", p=128)
    dct_r = dct_d.ap().rearrange("(b p) n -> p b n", p=128)
    out_r = out_d.ap().rearrange("(c p) f -> p c f", p=128)

    with ExitStack() as ctx:
        tc = ctx.enter_context(tile.TileContext(nc))
        consts = ctx.enter_context(tc.tile_pool(name="consts", bufs=1))
        xin = ctx.enter_context(tc.tile_pool(name="xin", bufs=1))
        proj = ctx.enter_context(tc.tile_pool(name="proj", bufs=1))
        pbuf = ctx.enter_context(tc.tile_pool(name="pbuf", bufs=1))
        rbuf = ctx.enter_context(tc.tile_pool(name="rbuf", bufs=8))
        ostage = ctx.enter_context(tc.tile_pool(name="ostage", bufs=1))
        psA = ctx.enter_context(tc.tile_pool(name="psA", bufs=4, space="PSUM"))
        psE = ctx.enter_context(tc.tile_pool(name="psE", bufs=4, space="PSUM"))

        # ---- DMA stream (single FIFO queue; order == consumption order) ----
        dctT_sb = consts.tile([128, NCH, M], BF)       # [n'-part, chunk, m]
        nc.sync.dma_start(dctT_sb[:], dctT_r)

        def quarters(name, src_r):
            tiles = []
            for qt in range(2 * NCH // QT):
                t = xin.tile([128, QT, HD], BF, tag=f"{name}{qt}")
                cs = slice(qt * QT, (qt + 1) * QT)
                nc.sync.dma_start(t[:], src_r[:, cs, :])
                tiles.append(t)
            return tiles

        q_t = quarters("q", q_r)   # chunks 0..7 = A-fold, 8..15 = B-fold
        k_t = quarters("k", k_r)
        v_t = quarters("v", v_r)

        dct_sb = consts.tile([128, MB, NH], BF)        # [m-part, m-block, n']
        nc.sync.dma_start(dct_sb[:], dct_r)

        # ---- persistent intermediates ----
        qdT_sb = proj.tile([128, HP, M], BF, tag="qdT")   # [2-head d, pair, m]
        kdT_sb = proj.tile([128, HP, M], BF, tag="kdT")
        vd_sb = proj.tile([128, MB, H, VW], BF, tag="vd")  # [m-part, mb, h, d+1]
        ctx_sb = proj.tile([128, MB, HD], BF, tag="ctx")   # [m-part, mb, h*d]
        ctxn_sb = proj.tile([128, HD], BF, tag="ctxn")     # -ctx odd block
        nc.vector.memset(vd_sb[:, :, :, D:VW], 1.0)
        ebias = consts.tile([128, 1], F32)
        nc.vector.memset(ebias[:], -4.0)

        def xc(tiles, c):  # folded chunk c (0..15), [128, HD]
            return tiles[c // QT][:, c % QT, :]

        # ---- Q/K projections: per parity phase, chunk-major over 6 groups --
        def proj_phase(tiles, dst_sb, par):
            # par 0: A chunks 0..7 against even-k dctT cols -> dst cols 0:128
            # par 1: B chunks 8..15 against odd-k cols     -> dst cols 128:256
            # Two passes (4 + 2 head-pairs) so only 4 PSUM banks are needed;
            # the second pass re-reads chunks already resident in SBUF.
            for hps in (range(4), range(4, HP)):
                groups = {
                    hp: psA.tile([128, 128], F32, tag="A", name=f"pg{par}{hp}")
                    for hp in hps
                }
                for c in range(NCH):
                    for hp in hps:
                        nc.tensor.matmul(
                            groups[hp][:],
                            lhsT=xc(tiles, par * NCH + c)[
                                :, hp * 128:(hp + 1) * 128
                            ],
                            rhs=dctT_sb[:, c, par * 128:(par + 1) * 128],
                            start=(c == 0),
                            stop=(c == NCH - 1),
                        )
                for hp in hps:
                    nc.vector.tensor_copy(
                        dst_sb[:, hp, par * 128:(par + 1) * 128], groups[hp][:]
                    )

        proj_phase(q_t, qdT_sb, 0)
        proj_phase(q_t, qdT_sb, 1)
        proj_phase(k_t, kdT_sb, 0)

        # ---- energy helper: one k-block of E^T + exp ----
        p_tiles = [None] * H

        def emit_energy(h, kb):
            hp, p0 = h // 2, (h % 2) * 64
            if p_tiles[h] is None:
                p_tiles[h] = pbuf.tile([128, MB, M], BF, tag=f"p{h}", name=f"p{h}")
            pe = psE.tile([128, M], F32, tag="E", name=f"e{h}{kb}")
            nc.tensor.matmul(
                pe[:],
                lhsT=kdT_sb[p0:p0 + 64, hp, kb * 128:(kb + 1) * 128],
                rhs=qdT_sb[p0:p0 + 64, hp, :],
                start=True,
                stop=True,
            )
            # P^T[k-block, m] = exp(E^T - 4); the -4 cancels in the
            # normalization and guards exp overflow for outlier logits.
            nc.scalar.activation(p_tiles[h][:, kb, :], pe[:], EXP, bias=ebias[:])

        # ---- even-k (kb=0) energy burst: needs only kdT even cols + qdT.
        # With psE bufs=4 the exps pipeline on ACT while the (DMA-paced)
        # K-proj B-phase matmuls run behind them on the PE. ----
        for h in range(H):
            emit_energy(h, 0)

        proj_phase(k_t, kdT_sb, 1)

        # ---- odd-k (kb=1) energy burst, then V-proj chases its DMAs ----
        for h in range(H):
            emit_energy(h, 1)

        # ---- V-proj interleaved with odd-k (kb=1) energies ----
        vgroups = [
            psA.tile([128, FW], F32, tag="A", name=f"vg{g}")
            for g in range(MB * FSPLIT)
        ]
        nhp = FW // D  # 6 heads per split
        for c in range(2 * NCH):   # c<8: A chunks -> even groups; c>=8: odd
            par = c // NCH
            for fs in range(FSPLIT):
                nc.tensor.matmul(
                    vgroups[par * FSPLIT + fs][:],
                    lhsT=dctT_sb[:, c % NCH, par * 128:(par + 1) * 128],
                    rhs=xc(v_t, c)[:, fs * FW:(fs + 1) * FW],
                    start=(c % NCH == 0),
                    stop=(c % NCH == NCH - 1),
                )
            if c % NCH == NCH - 1:   # evict this parity's Vd rows right away
                for fs in range(FSPLIT):
                    nc.vector.tensor_copy(
                        vd_sb[:, par, fs * nhp:(fs + 1) * nhp, 0:D],
                        vgroups[par * FSPLIT + fs][:].rearrange(
                            "p (h x) -> p h x", x=D
                        ),
                    )

        # ---- phase B: [ctx|S] psums; recip + scale on DVE (cheapest) ----
        def phase_b(h):
            p_t = p_tiles[h]
            for mb in range(MB):
                pc = psA.tile([128, VW], F32, tag="A", name=f"c{h}{mb}")
                for kb in range(MB):
                    nc.tensor.matmul(
                        pc[:],
                        lhsT=p_t[:, kb, mb * 128:(mb + 1) * 128],
                        rhs=vd_sb[:, kb, h, :],
                        start=(kb == 0),
                        stop=(kb == MB - 1),
                    )
                rs = rbuf.tile([128, 1], F32, tag="r", name=f"r{h}{mb}")
                nc.vector.reciprocal(rs[:], pc[:, D:VW])
                dst = ctx_sb[:, mb, h * D:(h + 1) * D]
                if h % 2 == 0:
                    nc.vector.tensor_scalar_mul(dst, pc[:, 0:D], rs[:])
                else:
                    nc.scalar.activation(dst, pc[:, 0:D], COPY, scale=rs[:])

        # ---- inverse DCT with 512/256 column splits (fewer psum copies);
        # lower half x[0:1024] = even+odd, upper y[j]=x[N-1-j] = even-odd ----
        FSW = (512, 256)

        def stage3_piece(fo, fw, act_share):
            fslice = slice(fo, fo + fw)
            NB = NH // 128  # 8 row-blocks per half
            for half in range(2):
                ost = ostage.tile(
                    [128, NB, fw], BF, tag=f"o{fo}{half}", name=f"o{fo}{half}"
                )
                for nb in range(NB):
                    px = psA.tile([128, fw], F32, tag="A", name=f"x{fo}{nb}")
                    nc.tensor.matmul(
                        px[:],
                        lhsT=dct_sb[:, 0, nb * 128:(nb + 1) * 128],
                        rhs=ctx_sb[:, 0, fslice],
                        start=True,
                        stop=False,
                    )
                    odd_rhs = ctx_sb[:, 1, fslice] if half == 0 else ctxn_sb[:, fslice]
                    nc.tensor.matmul(
                        px[:],
                        lhsT=dct_sb[:, 1, nb * 128:(nb + 1) * 128],
                        rhs=odd_rhs,
                        start=False,
                        stop=True,
                    )
                    if (half * NB + nb) % 4 < act_share:
                        nc.scalar.activation(ost[:, nb, :], px[:], COPY)
                    else:
                        nc.vector.tensor_copy(ost[:, nb, :], px[:])
                nc.sync.dma_start(
                    out_r[:, half * NB:(half + 1) * NB, fslice], ost[:]
                )

        for h in range(8):    # heads 0..7 cover ctx cols 0:512
            phase_b(h)
        nc.vector.tensor_scalar_mul(ctxn_sb[:, 0:512], ctx_sb[:, 1, 0:512], -1.0)
        for h in range(8, H):
            phase_b(h)
        nc.vector.tensor_scalar_mul(ctxn_sb[:, 512:HD], ctx_sb[:, 1, 512:HD], -1.0)
        # ACT carries most stage-3 copies: DVE already owns the normalize ops
        stage3_piece(0, 512, act_share=3)
        stage3_piece(512, 256, act_share=3)

    nc.compile()
    return nc


def prep_in_maps(Q, K, V, mask, Q_dct):
    Q, K, V = np.asarray(Q), np.asarray(K), np.asarray(V)
    mask, Q_dct = np.asarray(mask), np.asarray(Q_dct)
    scale = np.float32(1.0 / np.sqrt(np.sqrt(np.float32(D))))
    m4 = mask.astype(np.float32)[:, None, :, None]        # [B,1,N,1]

    def fold(x):  # [B,N,HD] -> [A; B] along N
        lo, hi = x[:, :NH, :], x[:, NH:, :][:, ::-1, :]
        return np.concatenate([lo + hi, lo - hi], axis=1)

    qs = fold((Q.astype(np.float32) * scale).transpose(0, 2, 1, 3).reshape(B, N, HD))
    ks = fold((K.astype(np.float32) * scale * m4).transpose(0, 2, 1, 3).reshape(B, N, HD))
    vs = fold((V.astype(np.float32) * m4).transpose(0, 2, 1, 3).reshape(B, N, HD))
    qs = np.ascontiguousarray(qs).astype(_BF16)
    ks = np.ascontiguousarray(ks).astype(_BF16)
    vs = np.ascontiguousarray(vs).astype(_BF16)

    dct_f = Q_dct.astype(np.float32)
    perm = np.concatenate([np.arange(0, M, 2), np.arange(1, M, 2)])
    dct_p = dct_f[perm]                            # rows reordered [even|odd]
    dct = np.ascontiguousarray(dct_p[:, :NH]).astype(_BF16)     # [M, NH]
    dctT = np.ascontiguousarray(dct_p[:, :NH].T).astype(_BF16)  # [NH, M]
    return [
        {"q": qs[b], "k": ks[b], "v": vs[b], "dctT": dctT, "dct": dct}
        for b in range(B)
    ]


def run(Q, K, V, mask, Q_dct, trace=False):
    from concourse.bass_utils import run_bass_kernel_spmd

    if "nc" not in _CACHE:
        _CACHE["nc"] = build_nc()
    nc = _CACHE["nc"]
    in_maps = prep_in_maps(Q, K, V, mask, Q_dct)
    res = run_bass_kernel_spmd(nc, in_maps, core_ids=list(range(B)), trace=trace)
    outs = np.stack(
        [res.results[i]["out"].astype(np.float32) for i in range(B)]
    )  # [B, N, HD]; rows NH: hold y[j] = x[N-1-j] -> un-reverse
    outs[:, NH:, :] = outs[:, NH:, :][:, ::-1, :]
    x = outs.reshape(B, N, H, D).transpose(0, 2, 1, 3)
    return np.ascontiguousarray(x, dtype=np.float32), res


def kernel(Q, K, V, mask, Q_dct):
    x, _ = run(Q, K, V, mask, Q_dct, trace=False)
    return x


# revision 31
# speedup vs baseline: 1.1486x; 1.1486x over previous
"""DCT-attention Trainium2 kernel (8 NeuronCores, data-parallel over batch).

Reference math (per b, h):
    Qd = dct @ (Q*s);  Kd = dct @ (K*s*mask);  Vd = dct @ (V*mask)   # [M,D]
    E  = Qd @ Kd^T;  P = softmax(E, axis=-1);  ctx = P @ Vd          # [M,D]
    x  = dct^T @ ctx                                                 # [N,D]
with B,H,N,D = 8,12,2048,64, M = 256, s = D**-0.25.

Sharding: batch b -> core b (8 cores). Host folds scale into Q/K and mask into
K/V, transposes to [N, H*D], bf16-casts; matmuls run bf16 -> fp32 PSUM; output
returns bf16 and is cast to f32 on the host.

DCT parity symmetry: dct[k, N-1-i] = (-1)^k dct[k, i].  The host therefore
uploads X folded as [A; B] with A = X[:N/2] + reverse(X[N/2:]),
B = X[:N/2] - reverse(X[N/2:]), and the M axis globally reordered to
[even k | odd k] (dctT columns, dct rows, and everything downstream use the
same order; softmax/ctx are permutation-invariant).  Every projection then
contracts over N/2 instead of N — half the matmul work.

Schedule per core (single FIFO DMA queue; PE chases the arrival stream):
  DMA:  dctT(0.5MB) | q | k | v | dct(1MB)
  PE:   Q-proj (A-phase 6 groups chunk-major, then B-phase) ->
        K-proj A -> K-proj B interleaved with even-k energies (exps on ACT
        overlap) -> V-proj interleaved with odd-k energies ->
        phase B heads 0-5 -> inverse-DCT fs0 half (+ out DMA) ->
        phase B heads 6-11 -> inverse-DCT fs1 half (+ out DMA).
"""

import numpy as np
import ml_dtypes

B, H, N, D = 8, 12, 2048, 64
M = 256
HD = H * D          # 768
NH = N // 2         # 1024 folded length
NCH = NH // 128     # 8 folded chunks per parity phase
QT = 4              # chunks per input-DMA quarter (of the folded [N, HD] array)
MB = M // 128       # 2 m-blocks (even ks | odd ks)
HP = H // 2         # 6 head-pairs
FSPLIT = 2          # HD split for <=512-wide psum
FW = HD // FSPLIT   # 384
VW = D + 1          # 65: Vd columns + ones column

_BF16 = ml_dtypes.bfloat16
_CACHE = {}


def build_nc():
    import concourse.bacc as bacc
    import concourse.mybir as mybir
    import concourse.tile as tile
    from contextlib import ExitStack

    BF = mybir.dt.bfloat16
    F32 = mybir.dt.float32
    EXP = mybir.ActivationFunctionType.Exp
    COPY = mybir.ActivationFunctionType.Copy

    # Bacc (not raw Bass): its compile() runs generate_event_semaphores,
    # which legalizes to this walrus's 1-sync-wait-per-instruction limit.
    nc = bacc.Bacc()
    q_d = nc.declare_dram_parameter("q", [N, HD], BF, isOutput=False)
    k_d = nc.declare_dram_parameter("k", [N, HD], BF, isOutput=False)
    v_d = nc.declare_dram_parameter("v", [N, HD], BF, isOutput=False)
    # dctT: [n' < N/2, m] with columns [even k | odd k]:
    #   cols 0:128  = dct[2j, n']^T,  cols 128:256 = dct[2j+1, n']^T
    dctT_d = nc.declare_dram_parameter("dctT", [NH, M], BF, isOutput=False)
    # dct: [M, N/2] rows reordered [even k | odd k]; the upper output half is
    # reconstructed from the same columns via x[N-1-i] = even - odd (host
    # un-reverses those rows).
    dct_d = nc.declare_dram_parameter("dct", [M, NH], BF, isOutput=False)
    out_d = nc.declare_dram_parameter("out", [N, HD], BF, isOutput=True)

    q_r = q_d.ap().rearrange("(c p) f -> p c f", p=128)
    k_r = k_d.ap().rearrange("(c p) f -> p c f", p=128)
    v_r = v_d.ap().rearrange("(c p) f -> p c f", p=128)
    dctT_r = dctT_d.ap().rearrange("(c p) m -> p c m", p=128)
    dct_r = dct_d.ap().rearrange("(b p) n -> p b n", p=128)
    out_r = out_d.ap().rearrange("(c p) f -> p c f", p=128)

    with ExitStack() as ctx:
        tc = ctx.enter_context(tile.TileContext(nc))
        consts = ctx.enter_context(tc.tile_pool(name="consts", bufs=1))
        xin = ctx.enter_context(tc.tile_pool(name="xin", bufs=1))
        proj = ctx.enter_context(tc.tile_pool(name="proj", bufs=1))
        pbuf = ctx.enter_context(tc.tile_pool(name="pbuf", bufs=1))
        rbuf = ctx.enter_context(tc.tile_pool(name="rbuf", bufs=8))
        ostage = ctx.enter_context(tc.tile_pool(name="ostage", bufs=1))
        psA = ctx.enter_context(tc.tile_pool(name="psA", bufs=6, space="PSUM"))
        psE = ctx.enter_context(tc.tile_pool(name="psE", bufs=2, space="PSUM"))

        # ---- DMA stream (single FIFO queue; order == consumption order) ----
        dctT_sb = consts.tile([128, NCH, M], BF)       # [n'-part, chunk, m]
        nc.sync.dma_start(dctT_sb[:], dctT_r)

        def quarters(name, src_r):
            tiles = []
            for qt in range(2 * NCH // QT):
                t = xin.tile([128, QT, HD], BF, tag=f"{name}{qt}")
                cs = slice(qt * QT, (qt + 1) * QT)
                nc.sync.dma_start(t[:], src_r[:, cs, :])
                tiles.append(t)
            return tiles

        q_t = quarters("q", q_r)   # chunks 0..7 = A-fold, 8..15 = B-fold
        k_t = quarters("k", k_r)
        v_t = quarters("v", v_r)

        dct_sb = consts.tile([128, MB, NH], BF)        # [m-part, m-block, n']
        nc.sync.dma_start(dct_sb[:], dct_r)

        # ---- persistent intermediates ----
        qdT_sb = proj.tile([128, HP, M], BF, tag="qdT")   # [2-head d, pair, m]
        kdT_sb = proj.tile([128, HP, M], BF, tag="kdT")
        vd_sb = proj.tile([128, MB, H, VW], BF, tag="vd")  # [m-part, mb, h, d+1]
        ctx_sb = proj.tile([128, MB, HD], BF, tag="ctx")   # [m-part, mb, h*d]
        ctxn_sb = proj.tile([128, HD], BF, tag="ctxn")     # -ctx odd block
        nc.vector.memset(vd_sb[:, :, :, D:VW], 1.0)
        ebias = consts.tile([128, 1], F32)
        nc.vector.memset(ebias[:], -4.0)

        def xc(tiles, c):  # folded chunk c (0..15), [128, HD]
            return tiles[c // QT][:, c % QT, :]

        # ---- Q/K projections: per parity phase, chunk-major over 6 groups --
        def proj_phase(tiles, dst_sb, par):
            # par 0: A chunks 0..7 against even-k dctT cols -> dst cols 0:128
            # par 1: B chunks 8..15 against odd-k cols     -> dst cols 128:256
            groups = [
                psA.tile([128, 128], F32, tag="A", name=f"pg{par}{hp}")
                for hp in range(HP)
            ]
            for c in range(NCH):
                for hp in range(HP):
                    nc.tensor.matmul(
                        groups[hp][:],
                        lhsT=xc(tiles, par * NCH + c)[:, hp * 128:(hp + 1) * 128],
                        rhs=dctT_sb[:, c, par * 128:(par + 1) * 128],
                        start=(c == 0),
                        stop=(c == NCH - 1),
                    )
            for hp in range(HP):
                nc.vector.tensor_copy(
                    dst_sb[:, hp, par * 128:(par + 1) * 128], groups[hp][:]
                )

        proj_phase(q_t, qdT_sb, 0)
        proj_phase(q_t, qdT_sb, 1)
        proj_phase(k_t, kdT_sb, 0)

        # ---- energy helper: one k-block of E^T + exp ----
        p_tiles = [None] * H

        def emit_energy(h, kb):
            hp, p0 = h // 2, (h % 2) * 64
            if p_tiles[h] is None:
                p_tiles[h] = pbuf.tile([128, MB, M], BF, tag=f"p{h}", name=f"p{h}")
            pe = psE.tile([128, M], F32, tag="E", name=f"e{h}{kb}")
            nc.tensor.matmul(
                pe[:],
                lhsT=kdT_sb[p0:p0 + 64, hp, kb * 128:(kb + 1) * 128],
                rhs=qdT_sb[p0:p0 + 64, hp, :],
                start=True,
                stop=True,
            )
            # P^T[k-block, m] = exp(E^T - 4); the -4 cancels in the
            # normalization and guards exp overflow for outlier logits.
            nc.scalar.activation(p_tiles[h][:, kb, :], pe[:], EXP, bias=ebias[:])

        # ---- K-proj B-phase interleaved with even-k (kb=0) energies ----
        kgroups = [
            psA.tile([128, 128], F32, tag="A", name=f"kg{hp}") for hp in range(HP)
        ]
        eq = [(h, 0) for h in range(H)]  # even-k energies, 2 per chunk: the
        for c in range(NCH):             # ACT exp chain is the serial resource
            for hp in range(HP):
                nc.tensor.matmul(
                    kgroups[hp][:],
                    lhsT=xc(k_t, NCH + c)[:, hp * 128:(hp + 1) * 128],
                    rhs=dctT_sb[:, c, 128:256],
                    start=(c == 0),
                    stop=(c == NCH - 1),
                )
            # kb0 energies only need kdT even columns (done) + qdT (done)
            for _ in range(2):
                if eq:
                    emit_energy(*eq.pop(0))
        while eq:
            emit_energy(*eq.pop(0))
        for hp in range(HP):
            nc.vector.tensor_copy(kdT_sb[:, hp, 128:256], kgroups[hp][:])

        # ---- V-proj interleaved with odd-k (kb=1) energies ----
        vgroups = [
            psA.tile([128, FW], F32, tag="A", name=f"vg{g}")
            for g in range(MB * FSPLIT)
        ]
        eq = [(h, 1) for h in range(H)]
        for c in range(2 * NCH):   # c<8: A chunks -> even groups; c>=8: odd
            par = c // NCH
            for fs in range(FSPLIT):
                nc.tensor.matmul(
                    vgroups[par * FSPLIT + fs][:],
                    lhsT=dctT_sb[:, c % NCH, par * 128:(par + 1) * 128],
                    rhs=xc(v_t, c)[:, fs * FW:(fs + 1) * FW],
                    start=(c % NCH == 0),
                    stop=(c % NCH == NCH - 1),
                )
            for _ in range(2):
                if eq:
                    emit_energy(*eq.pop(0))
            if c % NCH == NCH - 1:  # evict this parity's Vd rows right away
                nhp = FW // D  # 6 heads per split
                for fs in range(FSPLIT):
                    nc.vector.tensor_copy(
                        vd_sb[:, par, fs * nhp:(fs + 1) * nhp, 0:D],
                        vgroups[par * FSPLIT + fs][:].rearrange(
                            "p (h x) -> p h x", x=D
                        ),
                    )

        # ---- phase B + inverse DCT, split by fs half for earlier out DMA ---
        def phase_b(h):
            p_t = p_tiles[h]
            for mb in range(MB):
                pc = psA.tile([128, VW], F32, tag="A", name=f"c{h}{mb}")
                for kb in range(MB):
                    nc.tensor.matmul(
                        pc[:],
                        lhsT=p_t[:, kb, mb * 128:(mb + 1) * 128],
                        rhs=vd_sb[:, kb, h, :],
                        start=(kb == 0),
                        stop=(kb == MB - 1),
                    )
                rs = rbuf.tile([128, 1], F32, tag="r", name=f"r{h}{mb}")
                nc.vector.reciprocal(rs[:], pc[:, D:VW])
                dst = ctx_sb[:, mb, h * D:(h + 1) * D]
                if h % 2 == 0:
                    nc.vector.tensor_scalar_mul(dst, pc[:, 0:D], rs[:])
                else:
                    nc.scalar.activation(dst, pc[:, 0:D], COPY, scale=rs[:])

        def stage3_half(fs, extra=()):
            # half 0: output rows x[0:1024] = even + odd contributions.
            # half 1: y[j] = x[N-1-j] = even - odd (host un-reverses rows
            # 1024:, so we accumulate with the negated odd-block ctx).
            extra = list(extra)
            NB = NH // 128  # 8 row-blocks per half
            SUB = NB // 2   # 4-block out-DMA pieces: the last one starts early
            fslice = slice(fs * FW, (fs + 1) * FW)
            for half in range(2):
                for sub in range(2):
                    ost = ostage.tile(
                        [128, SUB, FW], BF,
                        tag=f"o{fs}{half}{sub}", name=f"o{fs}{half}{sub}",
                    )
                    for nbi in range(SUB):
                        nb = sub * SUB + nbi
                        px = psA.tile(
                            [128, FW], F32, tag="A", name=f"x{fs}{half}{nb}"
                        )
                        nc.tensor.matmul(
                            px[:],
                            lhsT=dct_sb[:, 0, nb * 128:(nb + 1) * 128],
                            rhs=ctx_sb[:, 0, fslice],
                            start=True,
                            stop=False,
                        )
                        odd_rhs = (
                            ctx_sb[:, 1, fslice] if half == 0
                            else ctxn_sb[:, fslice]
                        )
                        nc.tensor.matmul(
                            px[:],
                            lhsT=dct_sb[:, 1, nb * 128:(nb + 1) * 128],
                            rhs=odd_rhs,
                            start=False,
                            stop=True,
                        )
                        if nb % 2 == 0:
                            nc.vector.tensor_copy(ost[:, nbi, :], px[:])
                        else:
                            nc.scalar.activation(ost[:, nbi, :], px[:], COPY)
                        if extra:
                            extra.pop(0)()
                    nc.sync.dma_start(
                        out_r[
                            :,
                            half * NB + sub * SUB:half * NB + (sub + 1) * SUB,
                            fslice,
                        ],
                        ost[:],
                    )

        for h in range(6):   # heads 0..5 feed fs0
            phase_b(h)
        # negated odd-k ctx for the reconstructed upper output half
        nc.vector.tensor_scalar_mul(
            ctxn_sb[:, 0:FW], ctx_sb[:, 1, 0:FW], -1.0
        )
        # interleave heads 6..11 (and the fs1 negate) into the fs0 sweep so
        # the PE never waits on the normalize chain
        extra = [(lambda hh=h: phase_b(hh)) for h in range(6, H)]
        extra.append(
            lambda: nc.vector.tensor_scalar_mul(
                ctxn_sb[:, FW:HD], ctx_sb[:, 1, FW:HD], -1.0
            )
        )
        stage3_half(0, extra)
        stage3_half(1)

    nc.compile()
    return nc


def prep_in_maps(Q, K, V, mask, Q_dct):
    Q, K, V = np.asarray(Q), np.asarray(K), np.asarray(V)
    mask, Q_dct = np.asarray(mask), np.asarray(Q_dct)
    scale = np.float32(1.0 / np.sqrt(np.sqrt(np.float32(D))))
    m4 = mask.astype(np.float32)[:, None, :, None]        # [B,1,N,1]

    def fold(x):  # [B,N,HD] -> [A; B] along N
        lo, hi = x[:, :NH, :], x[:, NH:, :][:, ::-1, :]
        return np.concatenate([lo + hi, lo - hi], axis=1)

    qs = fold((Q.astype(np.float32) * scale).transpose(0, 2, 1, 3).reshape(B, N, HD))
    ks = fold((K.astype(np.float32) * scale * m4).transpose(0, 2, 1, 3).reshape(B, N, HD))
    vs = fold((V.astype(np.float32) * m4).transpose(0, 2, 1, 3).reshape(B, N, HD))
    qs = np.ascontiguousarray(qs).astype(_BF16)
    ks = np.ascontiguousarray(ks).astype(_BF16)
    vs = np.ascontiguousarray(vs).astype(_BF16)

    dct_f = Q_dct.astype(np.float32)
    perm = np.concatenate([np.arange(0, M, 2), np.arange(1, M, 2)])
    dct_p = dct_f[perm]                            # rows reordered [even|odd]
    dct = np.ascontiguousarray(dct_p[:, :NH]).astype(_BF16)     # [M, NH]
    dctT = np.ascontiguousarray(dct_p[:, :NH].T).astype(_BF16)  # [NH, M]
    return [
        {"q": qs[b], "k": ks[b], "v": vs[b], "dctT": dctT, "dct": dct}
        for b in range(B)
    ]


def run(Q, K, V, mask, Q_dct, trace=False):
    from concourse.bass_utils import run_bass_kernel_spmd

    if "nc" not in _CACHE:
        _CACHE["nc"] = build_nc()
    nc = _CACHE["nc"]
    in_maps = prep_in_maps(Q, K, V, mask, Q_dct)
    res = run_bass_kernel_spmd(nc, in_maps, core_ids=list(range(B)), trace=trace)
    outs = np.stack(
        [res.results[i]["out"].astype(np.float32) for i in range(B)]
    )  # [B, N, HD]; rows NH: hold y[j] = x[N-1-j] -> un-reverse
    outs[:, NH:, :] = outs[:, NH:, :][:, ::-1, :]
    x = outs.reshape(B, N, H, D).transpose(0, 2, 1, 3)
    return np.ascontiguousarray(x, dtype=np.float32), res


def kernel(Q, K, V, mask, Q_dct):
    x, _ = run(Q, K, V, mask, Q_dct, trace=False)
    return x
